# revision 1
# baseline (speedup 1.0000x reference)
"""Trainium2 Bass kernel v2 for nn_MultiHeadAttention_30374008717799.

Reference: q = k = v = x @ Wq.T, 16 heads of dim 64; causal softmax attention
with scale 1/sqrt(1024); output [B, S, 1024].

Sharding: 8 cores = 4 batches x 2 head-groups (8 heads each). Core gets x[b]
and its 512 rows of Wq, produces out[b, :, 512g:512g+512].

Design (HW ~192-210us/core vs 575us baseline; sim and HW agree within ~3%):
 - x and Wq are pre-transposed on the host during sharding, so the e
   contraction dim lands on SBUF partitions directly -- no on-chip x/wq
   transposes. Projection runs in f32r (full-rate fp32).
 - qT (d-major, bf16) feeds score matmuls with 2-head row-group packing at
   base partitions 0/64; qsd ([s,d] + ones column, bf16) via batched PE
   transposes feeds the AV matmuls (M=65: PSUM row 64 accumulates the
   softmax denominator Z for free).
 - Since k=v=q, exp'd score tiles are computed directly in [k,q]
   orientation (U is symmetric), so the AV moving operand needs no
   per-tile transposes.
 - Uniform N=512 score matmuls; exp in single [128,1024] ACT calls (bf16
   out); causal masking via gpsimd affine_select on minimal diagonal
   sub-ranges (Pool engine, otherwise idle). The mostly-masked final
   diagonal chunk pair is compacted to 384 live columns.
 - Per-head-pair projection/attention units are interleaved by an explicit
   schedule so PE prep work fills ACT-bound stretches; epilogue transposes
   ctxT back to [s,d] and normalizes by 1/Z on DVE.
 - NOTE: fp8+DoubleRow was tried and fails the 2e-2 gate (~4.5e-2):
   per-element q/prob noise passes straight to ctx with no sqrt-N
   averaging. gpsimd normalize_recip / load_library fail this walrus
   build ("ISA wrong length"), hence the DVE epilogue.
"""

import numpy as np

import concourse.bass as bass
import concourse.mybir as mybir
import concourse.tile as tile
from concourse.tile import ScopedClock
from concourse.bass_utils import run_bass_kernel_spmd

F32 = mybir.dt.float32
BF16 = mybir.dt.bfloat16
FP8 = mybir.dt.float8e4  # unused: fp8 fails the 2e-2 gate (no sqrt-N averaging of q noise)
AF = mybir.ActivationFunctionType
F32R = mybir.dt.float32r

S = 2048
E = 1024
DG = 512
D = 64
P = 128
SC = S // P        # 16
EC = E // P        # 8
DC = DG // P       # 4
WQ_PRESCALE = 1.0
SCALE = 1.0 / np.sqrt(1024.0) / (WQ_PRESCALE * WQ_PRESCALE)
ONES = WQ_PRESCALE

# feature flags (fallbacks if a primitive misbehaves on hardware)
USE_DR = False        # DoubleRow fp8 for proj + AV
USE_AFFSEL = True    # gpsimd affine_select for causal mask
USE_NRECIP = False    # gpsimd normalize_recip for the epilogue


class TC(tile.TileContext):
    """TileContext adapted to this walrus build, which caps sync-waits at ONE
    per instruction (extra waits peeled onto same-engine NoOps)."""

    MAX_WAITS = 1

    def _lower_ordered_insts(self, ordered):
        for bb_name, insts in ordered.items():
            new_list = []
            for inst in insts:
                si = inst.sync_info
                if si is not None and si.on_wait and len(si.on_wait) > 1:
                    waits = list(si.on_wait)
                    upds = list(si.on_update) if si.on_update else []
                    inst.sync_info = mybir.SyncInfo(
                        on_wait=waits[-1:], on_update=upds
                    )
                    for w in waits[:-1]:
                        nop = mybir.InstNoOp(
                            name=f"I-wsplit-{self.nc.next_id()}", ins=[], outs=[]
                        )
                        nop.engine = inst.engine
                        nop.sync_info = mybir.SyncInfo(on_wait=[w], on_update=[])
                        new_list.append(nop)
                new_list.append(inst)
            insts[:] = new_list
        return super()._lower_ordered_insts(ordered)

    def _drain_and_barrier(self, tick_clock, wait_clock):
        nc = self.nc
        drain_inst = nc.sync.drain()
        wait_clock.add_sem_waits(
            drain_inst.ins, ScopedClock({None: tick_clock.global_clock})
        )
        si = drain_inst.ins.sync_info
        waits = list(si.on_wait) if si is not None and si.on_wait else []
        upds = list(si.on_update) if si is not None and si.on_update else []
        if len(waits) > self.MAX_WAITS:
            drain_inst.ins.sync_info = mybir.SyncInfo(
                on_wait=waits[: self.MAX_WAITS], on_update=upds
            )
            rest = waits[self.MAX_WAITS:]
            for k in range(0, len(rest), self.MAX_WAITS):
                extra = nc.sync.drain()
                extra.ins.sync_info = mybir.SyncInfo(
                    on_wait=rest[k : k + self.MAX_WAITS], on_update=[]
                )
        nc.all_engine_barrier()
        popped = nc._tile_sem_poison_stack.pop()
        assert popped is self._sem_poison
        nc.clear_and_free_semaphores(list(self.sems.allocated().values()))
        nc.all_engine_barrier()


def build(reps=1):
    nc = bass.Bass("TRN2", target_bir_lowering=False, debug=False)
    xt_d = nc.declare_dram_parameter("xt", [E, S], F32R, isOutput=False)
    wqt_d = nc.declare_dram_parameter("wqt", [E, DG], F32R, isOutput=False)
    iden_d = nc.declare_dram_parameter("iden", [P, P], F32, isOutput=False)
    out_d = nc.declare_dram_parameter("out", [S, DG], F32, isOutput=True)

    from contextlib import ExitStack
    import contextlib

    with TC(nc) as tc, ExitStack() as es:
        cpool = es.enter_context(tc.tile_pool(name="consts", bufs=1))
        big = es.enter_context(tc.tile_pool(name="big", bufs=1))
        ut_pool = es.enter_context(tc.tile_pool(name="ut", bufs=8))
        ep_pool = es.enter_context(tc.tile_pool(name="ep", bufs=4))
        xt_pool = es.enter_context(tc.tile_pool(name="xt", bufs=2))
        psA = es.enter_context(tc.tile_pool(name="psA", bufs=2, space="PSUM"))
        psS = es.enter_context(tc.tile_pool(name="psS", bufs=2, space="PSUM"))
        psC = es.enter_context(tc.tile_pool(name="psC", bufs=2, space="PSUM"))

        if USE_NRECIP:
            from concourse import library_config
            nc.gpsimd.load_library(library_config.attn)

        # constants
        idf = cpool.tile([P, P], F32, name="idf")
        nc.sync.dma_start(idf[:], iden_d[:])
        idb = cpool.tile([P, P], BF16, name="idb")
        nc.vector.tensor_copy(idb[:], idf[:])

        loop_cm = tc.For_i(0, reps, 1) if reps > 1 else contextlib.nullcontext()
        es.enter_context(loop_cm)

        # persistent per-iteration tensors
        wqT = big.tile([P, EC, DG], F32R, name="wqT")          # [e, ec, d]
        qT = big.tile([P, DC, S], BF16, name="qT")            # [d(2h), dc, s]
        qsd = big.tile([P, DC, SC, 130], BF16, name="qsd")     # [s, dc, sc, 2x65]
        ctx_out = big.tile([P, SC, DG], F32, name="ctx_out")

        # ---- wqT and xT arrive pre-transposed from the host (the shard
        # layout choice): contraction dim e sits on partitions directly.
        # wq loads are per-dc so dc0 + the first x block gate the pipeline ----
        def load_wq(dc):
            nc.sync.dma_start(
                wqT[:, :, dc * P : (dc + 1) * P],
                wqt_d[:, dc * P : (dc + 1) * P].rearrange(
                    "(ec p) d -> p ec d", p=P
                ),
            )

        _xtb = {}

        def prep_x(sb):
            """DMA x block sb (pre-transposed) into xtb [e, ec, s-in-block]."""
            xtb = xt_pool.tile([P, EC, 512], F32R, name="xtb")
            _xtb[sb] = xtb
            for eh in range(4):
                nc.sync.dma_start(
                    xtb[:, 2 * eh : 2 * eh + 2, :],
                    xt_d[
                        eh * 256 : (eh + 1) * 256,
                        sb * 512 : (sb + 1) * 512,
                    ].rearrange("(ec p) s -> p ec s", p=P),
                )

        def prep_dc(sb, dc):
            """Project qT and qsd for head pair dc of x block sb."""
            xtb = _xtb[sb]
            pq = psA.tile([P, 512], F32, name="pq", tag="pt")
            for ec in range(EC):
                nc.tensor.matmul(
                    pq[:],
                    lhsT=wqT[:, ec, dc * P : (dc + 1) * P],
                    rhs=xtb[:, ec, :],
                    start=(ec == 0),
                    stop=(ec == EC - 1),
                )
            nc.vector.tensor_copy(
                qT[:, dc, sb * 512 : (sb + 1) * 512], pq[:]
            )
            # qsd via transposes of qT (4 chunks into one PSUM bank)
            ptq = psA.tile([P, 4, P], BF16, name="ptq", tag="pt")
            for j4 in range(4):
                j = 4 * sb + j4
                nc.tensor.transpose(
                    ptq[:, j4, :], qT[:, dc, j * P : (j + 1) * P], idb[:]
                )
            dst = qsd[:, dc, 4 * sb : 4 * sb + 4, :].rearrange(
                "p j (g c) -> p j g c", g=2
            )[:, :, :, 0:64]
            src = ptq[:].rearrange("p j (g c) -> p j g c", g=2)
            nc.vector.tensor_copy(dst, src)
            # ones columns feed PSUM row 64 = softmax denominator
            nc.vector.memset(
                qsd[:, dc, 4 * sb : 4 * sb + 4, :].rearrange(
                    "p j (g c) -> p j g c", g=2
                )[:, :, :, 64:65],
                ONES,
            )

        def attention_dc(i, dc, extra=None):
            """All-J attention for q-block i, head pair dc (heads 2dc, 2dc+1).

            Super-chunks J=0..2i-1 are fully-live pairs (chunks 2J, 2J+1).
            J=2i is the first diagonal pair (chunks 4i, 4i+1): full-width exp,
            then two small triangle masks. The final pair (chunks 4i+2, 4i+3)
            is COMPACTED: only their live columns (256+128) are scored/exp'd,
            packed into cols 0:384 of one tile.
            """
            cps = [
                psC.tile([P, 512], F32, name=f"cps{h2}", tag="cps")
                for h2 in range(2)
            ]
            for J in range(2 * i + 1):
                j0 = 2 * J
                for h2 in range(2):
                    pb = h2 * 64
                    st = psS.tile([P, 1024], F32, name="st", tag="st")
                    for u in range(2):
                        jj = j0 + u
                        nc.tensor.matmul(
                            st[:, u * 512 : (u + 1) * 512],
                            lhsT=qT[pb : pb + 64, dc, jj * P : (jj + 1) * P],
                            rhs=qT[pb : pb + 64, dc, i * 512 : (i + 1) * 512],
                            start=True,
                            stop=True,
                        )
                    ut = ut_pool.tile([P, 2, 512], BF16, name="ut")
                    nc.scalar.activation(
                        ut[:].rearrange("p u q -> p (u q)"), st[:],
                        AF.Exp, scale=SCALE,
                    )
                    if J == 2 * i:  # first diagonal pair: two triangle masks
                        nc.gpsimd.affine_select(
                            ut[:, 0, 0:P], ut[:, 0, 0:P],
                            pattern=[[1, P]],
                            compare_op=mybir.AluOpType.is_ge,
                            fill=0.0, base=0, channel_multiplier=-1,
                        )
                        nc.gpsimd.affine_select(
                            ut[:, 1, 0:256], ut[:, 1, 0:256],
                            pattern=[[1, 256]],
                            compare_op=mybir.AluOpType.is_ge,
                            fill=0.0, base=-P, channel_multiplier=-1,
                        )
                    for u in range(2):
                        cu = max(0, (j0 + u) * P - i * 512)
                        nc.tensor.matmul(
                            cps[h2][0:65, cu:512],
                            lhsT=qsd[:, dc, j0 + u, h2 * 65 : h2 * 65 + 65],
                            rhs=ut[:, u, cu:512],
                            start=(J == 0 and u == 0),
                            stop=False,
                        )
            # compact final pair: chunks 4i+2 (live q 256:512) and 4i+3
            # (live q 384:512) packed into cols 0:384 of one tile
            for h2 in range(2):
                pb = h2 * 64
                st = psS.tile([P, 1024], F32, name="st", tag="st")
                nc.tensor.matmul(
                    st[:, 0:256],
                    lhsT=qT[pb : pb + 64, dc, (4 * i + 2) * P : (4 * i + 3) * P],
                    rhs=qT[pb : pb + 64, dc, i * 512 + 256 : (i + 1) * 512],
                    start=True, stop=True,
                )
                nc.tensor.matmul(
                    st[:, 256:384],
                    lhsT=qT[pb : pb + 64, dc, (4 * i + 3) * P : (4 * i + 4) * P],
                    rhs=qT[pb : pb + 64, dc, i * 512 + 384 : (i + 1) * 512],
                    start=True, stop=True,
                )
                ut = ut_pool.tile([P, 2, 512], BF16, name="ut")
                nc.scalar.activation(ut[:, 0, 0:384], st[:, 0:384],
                                     AF.Exp, scale=SCALE)
                nc.gpsimd.affine_select(
                    ut[:, 0, 0:P], ut[:, 0, 0:P],
                    pattern=[[1, P]],
                    compare_op=mybir.AluOpType.is_ge,
                    fill=0.0, base=0, channel_multiplier=-1,
                )
                nc.gpsimd.affine_select(
                    ut[:, 0, 256:384], ut[:, 0, 256:384],
                    pattern=[[1, P]],
                    compare_op=mybir.AluOpType.is_ge,
                    fill=0.0, base=0, channel_multiplier=-1,
                )
                nc.tensor.matmul(
                    cps[h2][0:65, 256:512],
                    lhsT=qsd[:, dc, 4 * i + 2, h2 * 65 : h2 * 65 + 65],
                    rhs=ut[:, 0, 0:256],
                    start=False, stop=False,
                )
                nc.tensor.matmul(
                    cps[h2][0:65, 384:512],
                    lhsT=qsd[:, dc, 4 * i + 3, h2 * 65 : h2 * 65 + 65],
                    rhs=ut[:, 0, 256:384],
                    start=False, stop=True,
                )
            # prep work for the next block slots here: PE stays busy on it
            # while DVE drains the csb copies the epilogue transposes need
            if extra:
                for fn in extra:
                    fn()
            # epilogue: transpose ctxT back to [s, d], normalize by PSUM row 64
            for h2 in range(2):
                h = 2 * dc + h2
                csb = ep_pool.tile([65, 512], BF16, name="csb", tag="csb")
                nc.vector.tensor_copy(csb[:], cps[h2][0:65, :])
                ptc = psA.tile([P, 4, 66], BF16, name="ptc", tag="pt")
                for c in range(4):
                    nc.tensor.transpose(
                        ptc[:, c, 0:65], csb[:, c * P : (c + 1) * P], idb[0:65, 0:65]
                    )
                cs2 = ep_pool.tile([P, 4, 65], F32, name="cs2", tag="cs2")
                nc.vector.tensor_copy(cs2[:], ptc[:, :, 0:65])
                for c in range(4):
                    sc = 4 * i + c
                    dst = ctx_out[:, sc, h * D : (h + 1) * D]
                    if USE_NRECIP:
                        nc.gpsimd.normalize_recip(
                            dst, cs2[:, c, 0:64], cs2[:, c, 64:65]
                        )
                    else:
                        rc = ep_pool.tile([P, 1], F32, name="rc", tag="rc")
                        nc.vector.reciprocal(rc[:], cs2[:, c, 64:65])
                        nc.vector.tensor_scalar_mul(dst, cs2[:, c, 0:64], rc[:])

        # Emission schedule: prep units for block i+1 are woven into block i's
        # attention (between the AV tail and the epilogue of each dc) so the
        # PE never sits idle waiting on DVE epilogue copies. Block 0 starts
        # minimal (prep_x + dc0's projection) so the first exp lands early.
        def unit(fn, *args):
            return lambda: fn(*args)

        sched = {
            (0, 0): [unit(prep_dc, 0, 1)],
            (0, 1): [unit(prep_dc, 0, 2)],
            (0, 2): [unit(prep_dc, 0, 3), unit(prep_dc, 1, 0)],
            (0, 3): [unit(prep_dc, 1, 1)],
            (1, 0): [unit(prep_dc, 1, 2)],
            (1, 1): [unit(prep_dc, 1, 3)],
            (1, 2): [unit(prep_dc, 2, 0), unit(prep_dc, 2, 1)],
            (1, 3): [unit(prep_dc, 2, 2)],
            (2, 0): [unit(prep_dc, 2, 3)],
            (2, 2): [unit(prep_dc, 3, 0), unit(prep_dc, 3, 1)],
            (2, 3): [unit(prep_dc, 3, 2)],
            (3, 0): [unit(prep_dc, 3, 3)],
        }

        load_wq(0)
        prep_x(0)
        for dc in range(1, DC):
            load_wq(dc)
        prep_dc(0, 0)
        for i in range(4):
            if i < 3:
                prep_x(i + 1)
            for dc in range(DC):
                attention_dc(i, dc, extra=sched.get((i, dc)))
            for c in range(4):
                sc = 4 * i + c
                nc.sync.dma_start(
                    out_d[sc * P : (sc + 1) * P, :], ctx_out[:, sc, :]
                )

    return nc


def make_in_maps(x, Wq):
    iden = np.eye(P, dtype=np.float32)
    x = np.asarray(x, dtype=np.float32)
    Wq = np.asarray(Wq, dtype=np.float32)
    in_maps = []
    for c in range(8):
        b, g = c // 2, c % 2
        in_maps.append(
            {
                "xt": np.ascontiguousarray(x[b].T),
                "wqt": np.ascontiguousarray(Wq[g * DG : (g + 1) * DG].T),
                "iden": iden,
            }
        )
    return in_maps


_NC_CACHE = {}


def _get_nc():
    if "nc" not in _NC_CACHE:
        _NC_CACHE["nc"] = build()
    return _NC_CACHE["nc"]


def run(x, Wq, **spmd_kwargs):
    x = np.asarray(x, dtype=np.float32)
    Wq = np.asarray(Wq, dtype=np.float32)
    nc = _get_nc()
    in_maps = make_in_maps(x, Wq)
    kr = run_bass_kernel_spmd(nc, in_maps, list(range(8)), **spmd_kwargs)
    out = np.empty((4, S, E), dtype=np.float32)
    for c in range(8):
        b, g = c // 2, c % 2
        out[b, :, g * DG : (g + 1) * DG] = kr.results[c]["out"]
    return out.reshape(4, S, E), kr


def kernel(x, Wq):
    out, _ = run(x, Wq)
    return out



# revision 2
# speedup vs baseline: 1.2013x; 1.2013x over previous
"""Trainium2 Bass kernel v3 for nn_MultiHeadAttention_30374008717799.

Reference: q = k = v = x @ Wq.T, 16 heads of dim 64; causal softmax attention
with scale 1/sqrt(1024); output [B, S, 1024].

Sharding: 8 cores = 4 batches x 2 head-groups (8 heads each). Core gets x[b]
and its 512 rows of Wq (both pre-transposed AND pre-cast to bf16 on host),
produces out[b, :, 512g:512g+512].

v3 over v2 (v2: 207us sim, 210-400us HW depending on device state; v3:
189us sim, 182-269ns/rep paired-probe vs 240-498 for v2 same-run):
 - Globally software-pipelined emission: each J-step's scores+exp are
   emitted; its AV matmuls follow TWO steps later (deque), with unit
   epilogues riding after the next unit's first scores. Projection/
   transpose prep drips between J-steps as two coarse micro-tasks
   (whole-proj, transposes) so the shared 2-slab PSUM pool is never
   held across a drip boundary. Keeps ACT (the ~148us-busy bottleneck
   engine: 18.3M softmax exps at 128 lanes / 1.2 GHz) fed across unit
   boundaries.
 - The two heads' K=64 score matmuls are emitted interleaved
   [h0u0,h1u0,h0u1,h1u1]; base partitions 0/64 auto-derive PE row tiles
   (0,0)/(64,0) so adjacent MMs can overlap in disjoint array halves.
 - Compact (final diagonal) pair packs BOTH heads into one st slab
   (h0 cols 0:384, h1 cols 512:896, one 896-wide exp), freeing the
   other slab so the next unit's first scores start immediately.
 - x and Wq ship as bf16 (halves input DMA; ~0.3% proj noise, gate 2e-2).
 - Startup: ACT exp-table preloaded via dummy exp during the DMA phase;
   identity synthesized on-chip (no iden input); PE warmed with dummy
   matmuls so HAM un-throttles before the first projection; DMA queue
   ordered so the chunks gating the first proj matmuls land first.
 - NOTE (HW lessons): bufs=1 PSUM pools serialize badly on HW (sem
   round-trips the cost sim does not model) -- keep pq/ptq/ptc on one
   bufs=2 pool with allocations never spanning a drip boundary.
   Depth-2 AV deferral beats depth-1 AND depth-3 on HW. Device
   throughput drifts ~2x between sessions; only same-run paired
   probes are comparable.
 - NOTE: fp8+DoubleRow fails the 2e-2 gate (~4.5e-2): per-element
   q/prob noise passes straight to ctx with no sqrt-N averaging.
   gpsimd load_library fails this walrus build ("ISA wrong length").
"""

import numpy as np
from collections import deque

import concourse.bass as bass
import concourse.mybir as mybir
import concourse.tile as tile
from concourse.tile import ScopedClock
from concourse.bass_utils import run_bass_kernel_spmd

F32 = mybir.dt.float32
BF16 = mybir.dt.bfloat16
AF = mybir.ActivationFunctionType

S = 2048
E = 1024
DG = 512
D = 64
P = 128
SC = S // P        # 16
EC = E // P        # 8
DC = DG // P       # 4
SCALE = 1.0 / np.sqrt(1024.0)
ONES = 1.0


class TC(tile.TileContext):
    """TileContext adapted to this walrus build, which caps sync-waits at ONE
    per instruction (extra waits peeled onto same-engine NoOps)."""

    MAX_WAITS = 1

    def _lower_ordered_insts(self, ordered):
        for bb_name, insts in ordered.items():
            new_list = []
            for inst in insts:
                si = inst.sync_info
                if si is not None and si.on_wait and len(si.on_wait) > 1:
                    waits = list(si.on_wait)
                    upds = list(si.on_update) if si.on_update else []
                    inst.sync_info = mybir.SyncInfo(
                        on_wait=waits[-1:], on_update=upds
                    )
                    for w in waits[:-1]:
                        nop = mybir.InstNoOp(
                            name=f"I-wsplit-{self.nc.next_id()}", ins=[], outs=[]
                        )
                        nop.engine = inst.engine
                        nop.sync_info = mybir.SyncInfo(on_wait=[w], on_update=[])
                        new_list.append(nop)
                new_list.append(inst)
            insts[:] = new_list
        return super()._lower_ordered_insts(ordered)

    def _drain_and_barrier(self, tick_clock, wait_clock):
        nc = self.nc
        drain_inst = nc.sync.drain()
        wait_clock.add_sem_waits(
            drain_inst.ins, ScopedClock({None: tick_clock.global_clock})
        )
        si = drain_inst.ins.sync_info
        waits = list(si.on_wait) if si is not None and si.on_wait else []
        upds = list(si.on_update) if si is not None and si.on_update else []
        if len(waits) > self.MAX_WAITS:
            drain_inst.ins.sync_info = mybir.SyncInfo(
                on_wait=waits[: self.MAX_WAITS], on_update=upds
            )
            rest = waits[self.MAX_WAITS:]
            for k in range(0, len(rest), self.MAX_WAITS):
                extra = nc.sync.drain()
                extra.ins.sync_info = mybir.SyncInfo(
                    on_wait=rest[k : k + self.MAX_WAITS], on_update=[]
                )
        nc.all_engine_barrier()
        popped = nc._tile_sem_poison_stack.pop()
        assert popped is self._sem_poison
        nc.clear_and_free_semaphores(list(self.sems.allocated().values()))
        nc.all_engine_barrier()


def build(reps=1):
    nc = bass.Bass("TRN2", target_bir_lowering=False, debug=False)
    xt_d = nc.declare_dram_parameter("xt", [E, S], BF16, isOutput=False)
    wqt_d = nc.declare_dram_parameter("wqt", [E, DG], BF16, isOutput=False)
    out_d = nc.declare_dram_parameter("out", [S, DG], F32, isOutput=True)

    from contextlib import ExitStack
    import contextlib

    with TC(nc) as tc, ExitStack() as es:
        cpool = es.enter_context(tc.tile_pool(name="consts", bufs=1))
        big = es.enter_context(tc.tile_pool(name="big", bufs=1))
        ut_pool = es.enter_context(tc.tile_pool(name="ut", bufs=8))
        ep_pool = es.enter_context(tc.tile_pool(name="ep", bufs=4))
        xt_pool = es.enter_context(tc.tile_pool(name="xt", bufs=2))
        psA = es.enter_context(tc.tile_pool(name="psA", bufs=2, space="PSUM"))
        psS = es.enter_context(tc.tile_pool(name="psS", bufs=2, space="PSUM"))
        psC = es.enter_context(tc.tile_pool(name="psC", bufs=2, space="PSUM"))

        # constants: identity synthesized on-chip (memset 1 + diagonal
        # select), and a dummy exp to pull the ~2.7us ACT table load into
        # the input-DMA phase instead of the first real softmax tile.
        idb = cpool.tile([P, P], BF16, name="idb")
        nc.vector.memset(idb[:], 1.0)
        nc.gpsimd.affine_select(
            idb[:], idb[:],
            pattern=[[1, P]],
            compare_op=mybir.AluOpType.is_equal,
            fill=0.0, base=0, channel_multiplier=-1,
        )
        dume = cpool.tile([1, 2], F32, name="dume")
        nc.vector.memset(dume[:], 0.0)
        nc.scalar.activation(dume[:], dume[:], AF.Exp, scale=1.0)
        wups = psA.tile([P, P], F32, name="wup", tag="pt")
        for _ in range(36):
            nc.tensor.matmul(wups[:], lhsT=idb[:], rhs=idb[:], start=True, stop=True)

        loop_cm = tc.For_i(0, reps, 1) if reps > 1 else contextlib.nullcontext()
        es.enter_context(loop_cm)

        # persistent per-iteration tensors
        wqT = big.tile([P, EC, DG], BF16, name="wqT")          # [e, ec, d]
        qT = big.tile([P, DC, S], BF16, name="qT")             # [d(2h), dc, s]
        qsd = big.tile([P, DC, SC, 130], BF16, name="qsd")     # [s, dc, sc, 2x65]
        ctx_out = big.tile([P, SC, DG], F32, name="ctx_out")

        def load_wq(dc, eh=None):
            ecs = range(EC // 2) if eh == 0 else (
                range(EC // 2, EC) if eh == 1 else range(1)
            )
            if eh is None:
                nc.sync.dma_start(
                    wqT[:, :, dc * P : (dc + 1) * P],
                    wqt_d[:, dc * P : (dc + 1) * P].rearrange(
                        "(ec p) d -> p ec d", p=P
                    ),
                )
            else:
                e0 = 0 if eh == 0 else EC // 2
                e1 = EC // 2 if eh == 0 else EC
                nc.sync.dma_start(
                    wqT[:, e0:e1, dc * P : (dc + 1) * P],
                    wqt_d[e0 * P : e1 * P, dc * P : (dc + 1) * P].rearrange(
                        "(ec p) d -> p ec d", p=P
                    ),
                )

        _xtb = {}

        def prep_x_alloc(sb):
            xtb = xt_pool.tile([P, EC, 512], BF16, name="xtb")
            _xtb[sb] = xtb
            return xtb

        def prep_x_chunk(sb, eh):
            """DMA ec-pair chunk eh of pre-transposed bf16 x block sb."""
            nc.sync.dma_start(
                _xtb[sb][:, 2 * eh : 2 * eh + 2, :],
                xt_d[
                    eh * 256 : (eh + 1) * 256,
                    sb * 512 : (sb + 1) * 512,
                ].rearrange("(ec p) s -> p ec s", p=P),
            )

        def prep_x(sb):
            prep_x_alloc(sb)
            for eh in range(4):
                prep_x_chunk(sb, eh)

        # ---- prep micro-tasks: projection + qsd transposes for (sb, dc),
        # split into small closures so they drip between attention J-steps.
        def prep_micros(sb, dc):
            state = {}

            def m_proj(e0, e1):
                if e0 == 0:
                    state["pq"] = psA.tile([P, 512], F32, name="pq", tag="pt")
                pq = state["pq"]
                xtb = _xtb[sb]
                for ec in range(e0, e1):
                    nc.tensor.matmul(
                        pq[:],
                        lhsT=wqT[:, ec, dc * P : (dc + 1) * P],
                        rhs=xtb[:, ec, :],
                        start=(ec == 0),
                        stop=(ec == EC - 1),
                    )
                if e1 == EC:
                    nc.vector.tensor_copy(
                        qT[:, dc, sb * 512 : (sb + 1) * 512], pq[:]
                    )

            def m_tr():
                ptq = psA.tile([P, 4, P], BF16, name="ptq", tag="pt")
                for j4 in range(4):
                    j = 4 * sb + j4
                    nc.tensor.transpose(
                        ptq[:, j4, :], qT[:, dc, j * P : (j + 1) * P], idb[:]
                    )
                dst = qsd[:, dc, 4 * sb : 4 * sb + 4, :].rearrange(
                    "p j (g c) -> p j g c", g=2
                )[:, :, :, 0:64]
                src = ptq[:].rearrange("p j (g c) -> p j g c", g=2)
                nc.vector.tensor_copy(dst, src)
                nc.vector.memset(
                    qsd[:, dc, 4 * sb : 4 * sb + 4, :].rearrange(
                        "p j (g c) -> p j g c", g=2
                    )[:, :, :, 64:65],
                    ONES,
                )

            return [
                lambda: m_proj(0, 8),
                m_tr,
            ]

        # ---- attention unit (i, dc): built as J-steps, each a
        # (scores_and_exp_fn, av_fn) pair; plus an epilogue fn.
        def make_unit(i, dc):
            state = {"cps": None}

            def get_cps():
                if state["cps"] is None:
                    state["cps"] = [
                        psC.tile([P, 512], F32, name=f"cps{h2}", tag="cps")
                        for h2 in range(2)
                    ]
                return state["cps"]

            def make_full_step(J):
                # super-chunk pair J: k-chunks 2J, 2J+1 vs q-block i.
                j0 = 2 * J
                uts = {}

                def scores():
                    sts = {}
                    for h2 in range(2):
                        sts[h2] = psS.tile([P, 1024], F32, name="st", tag="st")
                    # interleave heads: row tiles (0,*) and (64,*) can
                    # overlap on HW when adjacent in the PE stream
                    for u in range(2):
                        for h2 in range(2):
                            pb = h2 * 64
                            nc.tensor.matmul(
                                sts[h2][:, u * 512 : (u + 1) * 512],
                                lhsT=qT[pb : pb + 64, dc, (j0 + u) * P : (j0 + u + 1) * P],
                                rhs=qT[pb : pb + 64, dc, i * 512 : (i + 1) * 512],
                                start=True,
                                stop=True,
                            )
                    for h2 in range(2):
                        ut = ut_pool.tile([P, 2, 512], BF16, name="ut")
                        uts[h2] = ut
                        nc.scalar.activation(
                            ut[:].rearrange("p u q -> p (u q)"), sts[h2][:],
                            AF.Exp, scale=SCALE,
                        )
                        if J == 2 * i:  # diagonal pair: two triangle masks
                            nc.gpsimd.affine_select(
                                ut[:, 0, 0:P], ut[:, 0, 0:P],
                                pattern=[[1, P]],
                                compare_op=mybir.AluOpType.is_ge,
                                fill=0.0, base=0, channel_multiplier=-1,
                            )
                            nc.gpsimd.affine_select(
                                ut[:, 1, 0:256], ut[:, 1, 0:256],
                                pattern=[[1, 256]],
                                compare_op=mybir.AluOpType.is_ge,
                                fill=0.0, base=-P, channel_multiplier=-1,
                            )

                def av():
                    cps = get_cps()
                    for h2 in range(2):
                        for u in range(2):
                            cu = max(0, (j0 + u) * P - i * 512)
                            nc.tensor.matmul(
                                cps[h2][0:65, cu:512],
                                lhsT=qsd[:, dc, j0 + u, h2 * 65 : h2 * 65 + 65],
                                rhs=uts[h2][:, u, cu:512],
                                start=(J == 0 and u == 0),
                                stop=False,
                            )

                return scores, av

            def make_compact_step():
                # final pair: chunks 4i+2 (live q 256:512) and 4i+3 (live q
                # 384:512), both heads packed into ONE st tile: h0 at cols
                # 0:384 (bank 0), h1 at cols 512:896 (bank 1). One exp call
                # covers 0:896 (the dead 384:512 strip is never read).
                # Using a single slab here frees the other psS slab so the
                # next unit's first scores can start without waiting.
                uts = {}

                def scores():
                    st = psS.tile([P, 1024], F32, name="st", tag="st")
                    for part in range(2):
                        for h2 in range(2):
                            pb = h2 * 64
                            cb = h2 * 512
                            if part == 0:
                                nc.tensor.matmul(
                                    st[:, cb : cb + 256],
                                    lhsT=qT[pb : pb + 64, dc, (4 * i + 2) * P : (4 * i + 3) * P],
                                    rhs=qT[pb : pb + 64, dc, i * 512 + 256 : (i + 1) * 512],
                                    start=True, stop=True,
                                )
                            else:
                                nc.tensor.matmul(
                                    st[:, cb + 256 : cb + 384],
                                    lhsT=qT[pb : pb + 64, dc, (4 * i + 3) * P : (4 * i + 4) * P],
                                    rhs=qT[pb : pb + 64, dc, i * 512 + 384 : (i + 1) * 512],
                                    start=True, stop=True,
                                )
                    ut = ut_pool.tile([P, 2, 512], BF16, name="ut")
                    uts[0] = ut
                    nc.scalar.activation(
                        ut[:].rearrange("p u q -> p (u q)")[:, 0:896],
                        st[:, 0:896], AF.Exp, scale=SCALE,
                    )
                    utf = ut[:].rearrange("p u q -> p (u q)")
                    for h2 in range(2):
                        cb = h2 * 512
                        nc.gpsimd.affine_select(
                            utf[:, cb : cb + P], utf[:, cb : cb + P],
                            pattern=[[1, P]],
                            compare_op=mybir.AluOpType.is_ge,
                            fill=0.0, base=0, channel_multiplier=-1,
                        )
                        nc.gpsimd.affine_select(
                            utf[:, cb + 256 : cb + 384], utf[:, cb + 256 : cb + 384],
                            pattern=[[1, P]],
                            compare_op=mybir.AluOpType.is_ge,
                            fill=0.0, base=0, channel_multiplier=-1,
                        )

                def av():
                    cps = get_cps()
                    utf = uts[0][:].rearrange("p u q -> p (u q)")
                    for h2 in range(2):
                        cb = h2 * 512
                        nc.tensor.matmul(
                            cps[h2][0:65, 256:512],
                            lhsT=qsd[:, dc, 4 * i + 2, h2 * 65 : h2 * 65 + 65],
                            rhs=utf[:, cb : cb + 256],
                            start=False, stop=False,
                        )
                        nc.tensor.matmul(
                            cps[h2][0:65, 384:512],
                            lhsT=qsd[:, dc, 4 * i + 3, h2 * 65 : h2 * 65 + 65],
                            rhs=utf[:, cb + 256 : cb + 384],
                            start=False, stop=True,
                        )

                return scores, av

            def epilogue():
                cps = get_cps()
                for h2 in range(2):
                    h = 2 * dc + h2
                    csb = ep_pool.tile([65, 512], BF16, name="csb", tag="csb")
                    nc.vector.tensor_copy(csb[:], cps[h2][0:65, :])
                    ptc = psA.tile([P, 4, 66], BF16, name="ptc", tag="pt")
                    for c in range(4):
                        nc.tensor.transpose(
                            ptc[:, c, 0:65], csb[:, c * P : (c + 1) * P],
                            idb[0:65, 0:65],
                        )
                    cs2 = ep_pool.tile([P, 4, 65], F32, name="cs2", tag="cs2")
                    nc.vector.tensor_copy(cs2[:], ptc[:, :, 0:65])
                    rc4 = ep_pool.tile([P, 4], F32, name="rc4", tag="rc")
                    nc.vector.reciprocal(
                        rc4[:], cs2[:, :, 64:65].rearrange("p c o -> p (c o)")
                    )
                    for c in range(4):
                        sc = 4 * i + c
                        nc.vector.tensor_scalar_mul(
                            ctx_out[:, sc, h * D : (h + 1) * D],
                            cs2[:, c, 0:64], rc4[:, c : c + 1],
                        )
                nc.sync.dma_start(
                    out_d[i * 512 : (i + 1) * 512, dc * P : (dc + 1) * P]
                    .rearrange("(c p) d -> p c d", p=P),
                    ctx_out[:, 4 * i : 4 * i + 4, dc * P : (dc + 1) * P],
                )

            steps = [make_full_step(J) for J in range(2 * i + 1)]
            steps.append(make_compact_step())
            return steps, epilogue

        # ---- global pipelined emission ----
        # startup DMA order matters: HWDGE dispatch is ~625ns per DMA, so
        # the chunks gating the first projection matmuls go out first,
        # interleaved so wq half 1 lands before proj needs ec4.
        prep_x_alloc(0)
        load_wq(0, eh=0)
        prep_x_chunk(0, 0)
        load_wq(0, eh=1)
        prep_x_chunk(0, 1)
        load_wq(1)
        prep_x_chunk(0, 2)
        prep_x_chunk(0, 3)
        load_wq(2)
        load_wq(3)

        prep_q = deque()
        for sb in range(4):
            for dc in range(DC):
                prep_q.append((sb, dc, prep_micros(sb, dc)))

        done_preps = set()

        def drip_prep(max_sb, n=1):
            """Emit up to n pending micro-prep tasks allowed at block max_sb."""
            k = 0
            while prep_q and k < n:
                sb0, dc0, micros = prep_q[0]
                if sb0 > max_sb:
                    break
                if micros:
                    micros.pop(0)()
                    k += 1
                if not micros:
                    done_preps.add((sb0, dc0))
                    prep_q.popleft()

        # prep (0,0) fully before the first unit
        sb0, dc0, micros = prep_q.popleft()
        for m in micros:
            m()
        done_preps.add((0, 0))

        # AV emission runs two J-steps behind its scores/exp; a unit's
        # epilogue rides with its last AV. This keeps ACT fed across unit
        # boundaries (the next unit's scores are emitted before the previous
        # unit's AV tail + epilogue).
        av_q = deque()  # entries: (av_fn, epi_fn_or_None)

        def pump(depth):
            while len(av_q) >= max(depth, 1):
                av_fn, epi_fn = av_q.popleft()
                av_fn()
                if epi_fn is not None:
                    epi_fn()
                if len(av_q) < depth:
                    break

        for i in range(4):
            if i < 3:
                prep_x(i + 1)
            for dc in range(DC):
                # required preps must be emitted before their readers
                while prep_q:
                    psb, pdc, micros = prep_q[0]
                    if (psb, pdc) in done_preps:
                        prep_q.popleft()
                    elif psb <= i and pdc <= dc:
                        for m in micros:
                            m()
                        done_preps.add((psb, pdc))
                        prep_q.popleft()
                    else:
                        break
                steps, epi = make_unit(i, dc)
                nsteps = len(steps)
                last_unit = (i, dc) == (3, DC - 1)
                for k, (scores_fn, av_fn) in enumerate(steps):
                    is_compact = k == nsteps - 1
                    scores_fn()
                    depth = 1 if (last_unit and k >= nsteps - 2) else 2
                    while len(av_q) >= depth:
                        av_fn0, epi_fn0 = av_q.popleft()
                        av_fn0()
                        if epi_fn0 is not None:
                            epi_fn0()
                    drip_prep(max_sb=min(i + 1, 3), n=1 if is_compact else 2)
                    av_q.append((av_fn, epi if is_compact else None))
        while av_q:
            av_fn0, epi_fn0 = av_q.popleft()
            av_fn0()
            if epi_fn0 is not None:
                epi_fn0()

    return nc


def make_in_maps(x, Wq):
    bf = mybir.dt.np(BF16)
    x = np.asarray(x, dtype=np.float32)
    Wq = np.asarray(Wq, dtype=np.float32)
    in_maps = []
    xts = [np.ascontiguousarray(x[b].T).astype(bf) for b in range(4)]
    wqts = [
        np.ascontiguousarray(Wq[g * DG : (g + 1) * DG].T).astype(bf)
        for g in range(2)
    ]
    for c in range(8):
        b, g = c // 2, c % 2
        in_maps.append({"xt": xts[b], "wqt": wqts[g]})
    return in_maps


_NC_CACHE = {}


def _get_nc():
    if "nc" not in _NC_CACHE:
        _NC_CACHE["nc"] = build()
    return _NC_CACHE["nc"]


def run(x, Wq, **spmd_kwargs):
    x = np.asarray(x, dtype=np.float32)
    Wq = np.asarray(Wq, dtype=np.float32)
    nc = _get_nc()
    in_maps = make_in_maps(x, Wq)
    kr = run_bass_kernel_spmd(nc, in_maps, list(range(8)), **spmd_kwargs)
    out = np.empty((4, S, E), dtype=np.float32)
    for c in range(8):
        b, g = c // 2, c % 2
        out[b, :, g * DG : (g + 1) * DG] = kr.results[c]["out"]
    return out.reshape(4, S, E), kr


def kernel(x, Wq):
    out, _ = run(x, Wq)
    return out
